# revision 2
# baseline (speedup 1.0000x reference)
"""Bayes classifier logits on 8 Trainium2 NeuronCores.

v2 (pipelined passes, persistent x/chunk-16 tiles) plus:
  - feature rows permuted on host so chunk 16's 32 square features are
    single-index (x_i^2): computed by a direct SBUF square of X^T rows
    (no sum-gen matmul, no PSUM evacuation for them)
  - one DVE-copy group's square runs on the otherwise-idle Pool engine
(The v3 col-tiled main matmul measured slower on HW and is reverted.)
"""

import numpy as np
import ml_dtypes

import concourse.bass as bass
from concourse import bacc, mybir, tile
from concourse.bass_utils import run_bass_kernel_spmd

B, C, D = 32768, 100, 64
N_CORES = 8
BS = B // N_CORES          # 4096 samples per core
NP_ = 512                  # samples per pass (one PSUM bank, fp32)
N_PASS = BS // NP_         # 8
N_PAIR = D * (D - 1) // 2  # 2016
N_FEAT = D + N_PAIR        # 2080 (singles first, then pairs i<j)
N_STORE = 18               # stored K-chunk slots (pad so chunks pair up 2x2)
N_CHUNK = 17               # K-chunks actually computed (2080 rows + 96 pad)
FEAT_PAD = N_STORE * 128   # 2304
N_GRP = N_STORE // 2       # 9 row-tiled chunk pairs
PHI_BUFS = 3               # phi pool buffers
SPSUM_BUFS = 3             # sum-gen PSUM tile buffers (2 banks each; 3*2+2=8)
OUT_BUFS = 2               # output staging buffers
# Per dual-group evacuation engine: 'A' = ACT fused Square, 'V' = DVE
# copy + DVE square, 'P' = DVE copy + Pool square.
EVAC = ['A', 'A', 'V', 'A', 'V', 'A', 'P', 'A']
SOLO_ENG = 'P'             # direct x_i^2 square engine ('V' DVE, 'P' Pool)
EPI_SPLIT = 256            # epilogue columns on ACT (rest on DVE)

_BF16 = mybir.dt.bfloat16
_F32 = mybir.dt.float32


def _host_prep(x, means, covs, weights):
    """Numpy (fp64) precompute of device weight operands."""
    mu = np.asarray(means).astype(np.float64)
    cv = np.asarray(covs).astype(np.float64)
    w = np.asarray(weights).astype(np.float64)

    L = np.linalg.cholesky(cv)                       # [C, D, D]
    logdet = 2.0 * np.sum(np.log(np.diagonal(L, axis1=1, axis2=2)), axis=1)
    P = np.linalg.inv(cv)                            # [C, D, D] (SPD)
    P = 0.5 * (P + np.transpose(P, (0, 2, 1)))
    q = np.einsum("cij,cj->ci", P, mu)               # [C, D]
    const = (np.log(w) - 0.5 * (logdet + D * np.log(2.0 * np.pi)
                                + np.einsum("ci,ci->c", mu, q)))

    iu, ju = np.triu_indices(D, k=1)                 # pair order (i<j)

    # E: [FEAT_PAD, D] 0/1 sum patterns.
    E = np.zeros((FEAT_PAD, D), dtype=np.float64)
    E[np.arange(D), np.arange(D)] = 1.0
    E[D + np.arange(N_PAIR), iu] = 1.0
    E[D + np.arange(N_PAIR), ju] = 1.0

    # Quadratic weights so that  sum_f Wq[f, c] * (E@x)_f^2 = -0.5 x^T P_c x
    Wq = np.zeros((FEAT_PAD, C), dtype=np.float64)
    Pij = P[:, iu, ju]                               # [C, N_PAIR]
    Wq[D + np.arange(N_PAIR), :] = (-0.5 * Pij).T
    Pdiag = np.diagonal(P, axis1=1, axis2=2)         # [C, D]
    offdiag_rowsum = P.sum(axis=2) - Pdiag
    Wq[np.arange(D), :] = (-0.5 * Pdiag + 0.5 * offdiag_rowsum).T

    # Linear + const terms folded into chunk 16's padding rows (the device
    # fills the matching phi rows with [ones; zeros; X^T]).  const is split
    # hi/lo across two ones-rows to survive the bf16 weight cast.
    c_hi = np.asarray(const.astype(ml_dtypes.bfloat16), dtype=np.float64)
    Wq[N_FEAT, :] = c_hi                             # row 2080: ones * hi
    Wq[N_FEAT + 1, :] = const - c_hi                 # row 2081: ones * lo
    Wq[N_FEAT + 32:N_FEAT + 96, :] = q.T             # rows 2112..2175: x_i

    # Permute features so chunk 16's square rows (2048..2079) are the
    # singles x_0..x_31: the device computes them by squaring X^T rows
    # directly in SBUF, so sum-gen only covers chunks 0..15.
    perm = np.concatenate([np.arange(32, 64), 64 + np.arange(N_PAIR),
                           np.arange(0, 32)])
    E[:N_FEAT] = E[perm]
    Wq[:N_FEAT] = Wq[perm]

    # Sum-gen stationary operands: lhsT_k = E[128k:128(k+1), :].T -> [64, 128]
    # stacked in pairs so chunk 2g+1 lives at SBUF partitions 64..127:
    # et_store[[0:64], g, :]  = lhsT_{2g},  et_store[[64:128], g, :] = lhsT_{2g+1}
    lhsT = E[:2048].reshape(16, 128, D).transpose(0, 2, 1)  # [16, 64, 128]
    et_store = np.concatenate(
        [lhsT[0::2], lhsT[1::2]], axis=1).transpose(1, 0, 2)  # [128, 8, 128]

    # Main-matmul stationary: wq_store[:, k, :] = Wq[128k:128(k+1), :]
    wq_store = Wq.reshape(N_STORE, 128, C).transpose(1, 0, 2)  # [128, 18, C]

    return {
        "et": np.ascontiguousarray(et_store).astype(ml_dtypes.bfloat16),
        "wq": np.ascontiguousarray(wq_store).astype(ml_dtypes.bfloat16),
    }


def _build_program(repeat=1):
    nc = bacc.Bacc("TRN2", target_bir_lowering=False, debug=False,
                   num_devices=N_CORES)
    xstack_d = nc.dram_tensor("xstack", [128, BS], _BF16,
                              kind="ExternalInput").ap()     # [X^T; X^T] bf16
    et_d = nc.dram_tensor("et", [128, 8, 128], _BF16,
                          kind="ExternalInput").ap()
    wq_d = nc.dram_tensor("wq", [128, N_STORE, C], _BF16,
                          kind="ExternalInput").ap()
    out_d = nc.dram_tensor("logits_t", [C, BS], _F32,
                           kind="ExternalOutput").ap()

    with tile.TileContext(nc) as tc:  # noqa: PLR1702
        with (
            tc.tile_pool(name="const", bufs=1) as cpool,
            tc.tile_pool(name="phi", bufs=PHI_BUFS) as phipool,
            tc.tile_pool(name="outp", bufs=OUT_BUFS) as opool,
            tc.tile_pool(name="psum_s", bufs=SPSUM_BUFS, space="PSUM") as spsum,
            tc.tile_pool(name="psum_o", bufs=2, space="PSUM") as opsum,
        ):
            et_t = cpool.tile([128, 8, 128], _BF16)
            nc.sync.dma_start(et_t[:], et_d[:])
            wq_t = cpool.tile([128, N_STORE, C], _BF16)
            nc.sync.dma_start(wq_t[:], wq_d[:])
            xall = cpool.tile([128, BS], _BF16)
            nc.sync.dma_start(xall[:], xstack_d[:])
            # Persistent chunk-16 phi: rows 0-31 per-pass squares; 32/33
            # ones (const hi/lo); 34-63 zero; 64-127 X^T (linear term).
            phi16 = cpool.tile([128, N_PASS, NP_], _BF16)
            nc.gpsimd.memset(phi16[32:64, :, :], 0.0)
            nc.gpsimd.memset(phi16[32:34, :, :], 1.0)
            nc.sync.dma_start(phi16[64:128, :, :], xstack_d[0:64, :])

            def emit_sumgen(p):
                """Sum-gen + square evacuation for pass p; returns phis."""
                ns = bass.ts(p, NP_)
                # chunk 16's square rows are singles x_0..x_31: square
                # X^T rows directly in SBUF, no matmul / PSUM round-trip
                sq_eng = nc.gpsimd if SOLO_ENG == 'P' else nc.vector
                sq_eng.tensor_tensor(
                    phi16[0:32, p, :], xall[0:32, ns], xall[0:32, ns],
                    mybir.AluOpType.mult)

                phis = [None] * 8
                for g in range(8):
                    phig = phipool.tile([128, 2, NP_], _BF16, tag=f"phi{g}")
                    phis[g] = phig
                    s2 = spsum.tile([128, 2, NP_], _F32, tag="s")
                    nc.tensor.matmul(s2[:, 0, :], et_t[0:64, g, :],
                                     xall[0:64, ns])
                    nc.tensor.matmul(s2[:, 1, :], et_t[64:128, g, :],
                                     xall[64:128, ns])
                    ev = EVAC[g]
                    if ev in ('V', 'P'):
                        # copy PSUM->SBUF bf16 on DVE, square in SBUF on
                        # DVE (2x bf16 rate) or the idle Pool engine
                        tmp = phipool.tile([128, 2, NP_], _BF16,
                                           tag=f"tmp{g}")
                        nc.vector.tensor_copy(tmp[:, 0:2, :], s2[:, 0:2, :])
                        sq = nc.gpsimd if ev == 'P' else nc.vector
                        sq.tensor_tensor(
                            phig[:, 0:2, :], tmp[:, 0:2, :], tmp[:, 0:2, :],
                            mybir.AluOpType.mult)
                    else:
                        nc.scalar.activation(
                            phig[:, 0:2, :], s2[:, 0:2, :],
                            mybir.ActivationFunctionType.Square)
                return phis

            def emit_main(p, phis):
                """Main accumulation + epilogue + output DMA for pass p."""
                ns = bass.ts(p, NP_)
                acc = opsum.tile([C, NP_], _F32, tag="acc")
                for k in range(N_CHUNK):
                    rhs = (phis[k // 2][:, k % 2, :] if k < 16
                           else phi16[:, p, :])
                    nc.tensor.matmul(acc[:], wq_t[:, k, :], rhs,
                                     start=(k == 0), stop=(k == N_CHUNK - 1))
                ot = opool.tile([C, NP_], _F32, tag="ot")
                if EPI_SPLIT > 0:
                    nc.scalar.copy(ot[:, 0:EPI_SPLIT], acc[:, 0:EPI_SPLIT])
                if EPI_SPLIT < NP_:
                    nc.vector.tensor_copy(ot[:, EPI_SPLIT:],
                                          acc[:, EPI_SPLIT:])
                nc.sync.dma_start(out_d[:, ns], ot[:])

            # Software pipeline: sum-gen for pass p+1 is emitted before
            # main for pass p, so ACT/DVE evacuation overlaps the main
            # matmul instead of idling through it.
            pend = None  # (pass, phis)
            for it in range(N_PASS * repeat):
                p = it % N_PASS
                phis = emit_sumgen(p)
                if pend is not None:
                    emit_main(*pend)
                pend = (p, phis)
            emit_main(*pend)

    nc.compile()
    return nc


_NC_CACHE = None


def _get_nc():
    global _NC_CACHE
    if _NC_CACHE is None:
        _NC_CACHE = _build_program()
    return _NC_CACHE


def _make_in_maps(x, prep):
    x = np.asarray(x)
    in_maps = []
    for c in range(N_CORES):
        xs = x[c * BS:(c + 1) * BS].astype(np.float32)     # [BS, D]
        xt = np.ascontiguousarray(xs.T)                    # [D, BS]
        xstack = np.concatenate([xt, xt], axis=0)
        in_maps.append({
            "xstack": np.ascontiguousarray(xstack.astype(ml_dtypes.bfloat16)),
            "et": prep["et"],
            "wq": prep["wq"],
        })
    return in_maps


def kernel(x, means, covs, weights):
    x = np.asarray(x)
    prep = _host_prep(x, means, covs, weights)
    nc = _get_nc()
    res = run_bass_kernel_spmd(nc, _make_in_maps(x, prep),
                               list(range(N_CORES)))
    outs = [res.results[c]["logits_t"] for c in range(N_CORES)]  # [C, BS]
    logits_t = np.concatenate(outs, axis=1)                      # [C, B]
    return np.ascontiguousarray(logits_t.T.astype(np.float32))   # [B, C]


# revision 3
# speedup vs baseline: 1.0156x; 1.0156x over previous
"""Bayes classifier logits on 8 Trainium2 NeuronCores.

logits[b, c] = log w_c - 0.5 * (maha_cb + logdet_c + D*log(2pi)),
maha_cb = (x_b - mu_c)^T P_c (x_b - mu_c), P_c = covs_c^{-1}.
Data-parallel over batch (8 cores); per core the quadratic term is one
PSUM-accumulated matmul over "squared-sum" features, using
x_i x_j = ((x_i+x_j)^2 - x_i^2 - x_j^2)/2 folded into host-precomputed
weights: S = E @ X^T (PE), Phi = S^2 (ACT/DVE/Pool during PSUM->SBUF
evacuation, bf16), acc = sum_k Wq_k^T Phi_k (PE, PSUM accumulate).

Schedule/structure (each measured faster on HW than the alternative):
  - passes are software-pipelined: sum-gen for pass p+1 is emitted
    before the main matmul of pass p, so the ACT/DVE/Pool square
    evacuation overlaps the main accumulation instead of idling
  - x and chunk-16's constant phi rows (ones/zeros/X^T) live in
    persistent SBUF tiles, loaded/memset once outside the loop
  - feature rows are permuted on host so chunk 16's 32 square rows are
    singles x_0..x_31^2, computed by a direct SBUF square on Pool (no
    sum-gen matmul / PSUM evacuation for them)
  - square evacuation is split ACT(5 groups, fused Square) /
    DVE(2 groups, copy+square) / Pool(1 group square + singles), the
    epilogue splits ACT/DVE; the linear/const terms ride as extra rows
    of chunk 16 (const split hi/lo across two bf16 weight rows)
"""

import numpy as np
import ml_dtypes

import concourse.bass as bass
from concourse import bacc, mybir, tile
from concourse.bass_utils import run_bass_kernel_spmd

B, C, D = 32768, 100, 64
N_CORES = 8
BS = B // N_CORES          # 4096 samples per core
NP_ = 512                  # samples per pass (one PSUM bank, fp32)
N_PASS = BS // NP_         # 8
N_PAIR = D * (D - 1) // 2  # 2016
N_FEAT = D + N_PAIR        # 2080 (singles first, then pairs i<j)
N_STORE = 18               # stored K-chunk slots (pad so chunks pair up 2x2)
N_CHUNK = 17               # K-chunks actually computed (2080 rows + 96 pad)
FEAT_PAD = N_STORE * 128   # 2304
N_GRP = N_STORE // 2       # 9 row-tiled chunk pairs
PHI_BUFS = 3               # phi pool buffers
SPSUM_BUFS = 3             # sum-gen PSUM tile buffers (2 banks each; 3*2+2=8)
OUT_BUFS = 2               # output staging buffers
# Per dual-group evacuation engine: 'A' = ACT fused Square, 'V' = DVE
# copy + DVE square, 'P' = DVE copy + Pool square.
EVAC = ['A', 'A', 'V', 'A', 'V', 'A', 'P', 'A']
SOLO_ENG = 'P'             # direct x_i^2 square engine ('V' DVE, 'P' Pool)
EPI_SPLIT = 256            # epilogue columns on ACT (rest on DVE)

_BF16 = mybir.dt.bfloat16
_F32 = mybir.dt.float32


def _host_prep(x, means, covs, weights):
    """Numpy (fp64) precompute of device weight operands."""
    mu = np.asarray(means).astype(np.float64)
    cv = np.asarray(covs).astype(np.float64)
    w = np.asarray(weights).astype(np.float64)

    L = np.linalg.cholesky(cv)                       # [C, D, D]
    logdet = 2.0 * np.sum(np.log(np.diagonal(L, axis1=1, axis2=2)), axis=1)
    P = np.linalg.inv(cv)                            # [C, D, D] (SPD)
    P = 0.5 * (P + np.transpose(P, (0, 2, 1)))
    q = np.einsum("cij,cj->ci", P, mu)               # [C, D]
    const = (np.log(w) - 0.5 * (logdet + D * np.log(2.0 * np.pi)
                                + np.einsum("ci,ci->c", mu, q)))

    iu, ju = np.triu_indices(D, k=1)                 # pair order (i<j)

    # E: [FEAT_PAD, D] 0/1 sum patterns.
    E = np.zeros((FEAT_PAD, D), dtype=np.float64)
    E[np.arange(D), np.arange(D)] = 1.0
    E[D + np.arange(N_PAIR), iu] = 1.0
    E[D + np.arange(N_PAIR), ju] = 1.0

    # Quadratic weights so that  sum_f Wq[f, c] * (E@x)_f^2 = -0.5 x^T P_c x
    Wq = np.zeros((FEAT_PAD, C), dtype=np.float64)
    Pij = P[:, iu, ju]                               # [C, N_PAIR]
    Wq[D + np.arange(N_PAIR), :] = (-0.5 * Pij).T
    Pdiag = np.diagonal(P, axis1=1, axis2=2)         # [C, D]
    offdiag_rowsum = P.sum(axis=2) - Pdiag
    Wq[np.arange(D), :] = (-0.5 * Pdiag + 0.5 * offdiag_rowsum).T

    # Linear + const terms folded into chunk 16's padding rows (the device
    # fills the matching phi rows with [ones; zeros; X^T]).  const is split
    # hi/lo across two ones-rows to survive the bf16 weight cast.
    c_hi = np.asarray(const.astype(ml_dtypes.bfloat16), dtype=np.float64)
    Wq[N_FEAT, :] = c_hi                             # row 2080: ones * hi
    Wq[N_FEAT + 1, :] = const - c_hi                 # row 2081: ones * lo
    Wq[N_FEAT + 32:N_FEAT + 96, :] = q.T             # rows 2112..2175: x_i

    # Permute features so chunk 16's square rows (2048..2079) are the
    # singles x_0..x_31: the device computes them by squaring X^T rows
    # directly in SBUF, so sum-gen only covers chunks 0..15.
    perm = np.concatenate([np.arange(32, 64), 64 + np.arange(N_PAIR),
                           np.arange(0, 32)])
    E[:N_FEAT] = E[perm]
    Wq[:N_FEAT] = Wq[perm]

    # Sum-gen stationary operands: lhsT_k = E[128k:128(k+1), :].T -> [64, 128]
    # stacked in pairs so chunk 2g+1 lives at SBUF partitions 64..127:
    # et_store[[0:64], g, :]  = lhsT_{2g},  et_store[[64:128], g, :] = lhsT_{2g+1}
    lhsT = E[:2048].reshape(16, 128, D).transpose(0, 2, 1)  # [16, 64, 128]
    et_store = np.concatenate(
        [lhsT[0::2], lhsT[1::2]], axis=1).transpose(1, 0, 2)  # [128, 8, 128]

    # Main-matmul stationary: wq_store[:, k, :] = Wq[128k:128(k+1), :]
    wq_store = Wq.reshape(N_STORE, 128, C).transpose(1, 0, 2)  # [128, 18, C]

    return {
        "et": np.ascontiguousarray(et_store).astype(ml_dtypes.bfloat16),
        "wq": np.ascontiguousarray(wq_store).astype(ml_dtypes.bfloat16),
    }


def _build_program(repeat=1):
    nc = bacc.Bacc("TRN2", target_bir_lowering=False, debug=False,
                   num_devices=N_CORES)
    xstack_d = nc.dram_tensor("xstack", [128, BS], _BF16,
                              kind="ExternalInput").ap()     # [X^T; X^T] bf16
    et_d = nc.dram_tensor("et", [128, 8, 128], _BF16,
                          kind="ExternalInput").ap()
    wq_d = nc.dram_tensor("wq", [128, N_STORE, C], _BF16,
                          kind="ExternalInput").ap()
    out_d = nc.dram_tensor("logits_t", [C, BS], _F32,
                           kind="ExternalOutput").ap()

    with tile.TileContext(nc) as tc:  # noqa: PLR1702
        with (
            tc.tile_pool(name="const", bufs=1) as cpool,
            tc.tile_pool(name="phi", bufs=PHI_BUFS) as phipool,
            tc.tile_pool(name="outp", bufs=OUT_BUFS) as opool,
            tc.tile_pool(name="psum_s", bufs=SPSUM_BUFS, space="PSUM") as spsum,
            tc.tile_pool(name="psum_o", bufs=2, space="PSUM") as opsum,
        ):
            et_t = cpool.tile([128, 8, 128], _BF16)
            nc.sync.dma_start(et_t[:], et_d[:])
            wq_t = cpool.tile([128, N_STORE, C], _BF16)
            nc.sync.dma_start(wq_t[:], wq_d[:])
            xall = cpool.tile([128, BS], _BF16)
            nc.sync.dma_start(xall[:], xstack_d[:])
            # Persistent chunk-16 phi: rows 0-31 per-pass squares; 32/33
            # ones (const hi/lo); 34-63 zero; 64-127 X^T (linear term).
            phi16 = cpool.tile([128, N_PASS, NP_], _BF16)
            nc.gpsimd.memset(phi16[32:64, :, :], 0.0)
            nc.gpsimd.memset(phi16[32:34, :, :], 1.0)
            nc.sync.dma_start(phi16[64:128, :, :], xstack_d[0:64, :])

            def emit_sumgen(p):
                """Sum-gen + square evacuation for pass p; returns phis."""
                ns = bass.ts(p, NP_)
                # chunk 16's square rows are singles x_0..x_31: square
                # X^T rows directly in SBUF, no matmul / PSUM round-trip
                sq_eng = nc.gpsimd if SOLO_ENG == 'P' else nc.vector
                sq_eng.tensor_tensor(
                    phi16[0:32, p, :], xall[0:32, ns], xall[0:32, ns],
                    mybir.AluOpType.mult)

                phis = [None] * 8
                for g in range(8):
                    phig = phipool.tile([128, 2, NP_], _BF16, tag=f"phi{g}")
                    phis[g] = phig
                    s2 = spsum.tile([128, 2, NP_], _F32, tag="s")
                    nc.tensor.matmul(s2[:, 0, :], et_t[0:64, g, :],
                                     xall[0:64, ns])
                    nc.tensor.matmul(s2[:, 1, :], et_t[64:128, g, :],
                                     xall[64:128, ns])
                    ev = EVAC[g]
                    if ev in ('V', 'P'):
                        # copy PSUM->SBUF bf16 on DVE, square in SBUF on
                        # DVE (2x bf16 rate) or the idle Pool engine
                        tmp = phipool.tile([128, 2, NP_], _BF16,
                                           tag=f"tmp{g}")
                        nc.vector.tensor_copy(tmp[:, 0:2, :], s2[:, 0:2, :])
                        sq = nc.gpsimd if ev == 'P' else nc.vector
                        sq.tensor_tensor(
                            phig[:, 0:2, :], tmp[:, 0:2, :], tmp[:, 0:2, :],
                            mybir.AluOpType.mult)
                    else:
                        nc.scalar.activation(
                            phig[:, 0:2, :], s2[:, 0:2, :],
                            mybir.ActivationFunctionType.Square)
                return phis

            def emit_main(p, phis):
                """Main accumulation + epilogue + output DMA for pass p."""
                ns = bass.ts(p, NP_)
                acc = opsum.tile([C, NP_], _F32, tag="acc")
                for k in range(N_CHUNK):
                    rhs = (phis[k // 2][:, k % 2, :] if k < 16
                           else phi16[:, p, :])
                    nc.tensor.matmul(acc[:], wq_t[:, k, :], rhs,
                                     start=(k == 0), stop=(k == N_CHUNK - 1))
                ot = opool.tile([C, NP_], _F32, tag="ot")
                if EPI_SPLIT > 0:
                    nc.scalar.copy(ot[:, 0:EPI_SPLIT], acc[:, 0:EPI_SPLIT])
                if EPI_SPLIT < NP_:
                    nc.vector.tensor_copy(ot[:, EPI_SPLIT:],
                                          acc[:, EPI_SPLIT:])
                nc.sync.dma_start(out_d[:, ns], ot[:])

            # Software pipeline: sum-gen for pass p+1 is emitted before
            # main for pass p, so ACT/DVE evacuation overlaps the main
            # matmul instead of idling through it.
            pend = None  # (pass, phis)
            for it in range(N_PASS * repeat):
                p = it % N_PASS
                phis = emit_sumgen(p)
                if pend is not None:
                    emit_main(*pend)
                pend = (p, phis)
            emit_main(*pend)

    nc.compile()
    return nc


_NC_CACHE = None


def _get_nc():
    global _NC_CACHE
    if _NC_CACHE is None:
        _NC_CACHE = _build_program()
    return _NC_CACHE


def _make_in_maps(x, prep):
    x = np.asarray(x)
    in_maps = []
    for c in range(N_CORES):
        xs = x[c * BS:(c + 1) * BS].astype(np.float32)     # [BS, D]
        xt = np.ascontiguousarray(xs.T)                    # [D, BS]
        xstack = np.concatenate([xt, xt], axis=0)
        in_maps.append({
            "xstack": np.ascontiguousarray(xstack.astype(ml_dtypes.bfloat16)),
            "et": prep["et"],
            "wq": prep["wq"],
        })
    return in_maps


def kernel(x, means, covs, weights):
    x = np.asarray(x)
    prep = _host_prep(x, means, covs, weights)
    nc = _get_nc()
    res = run_bass_kernel_spmd(nc, _make_in_maps(x, prep),
                               list(range(N_CORES)))
    outs = [res.results[c]["logits_t"] for c in range(N_CORES)]  # [C, BS]
    logits_t = np.concatenate(outs, axis=1)                      # [C, B]
    return np.ascontiguousarray(logits_t.T.astype(np.float32))   # [B, C]
